# revision 12
# baseline (speedup 1.0000x reference)
"""Trainium2 Bass kernel for AttentionalAggregation (segment softmax-weighted sum).

reference math:
    s = values @ gate_w + gate_b            # [N,1]
    w = segment_softmax(s, indices)         # [N,1]
    out = segment_sum(w * (values @ attn_w + attn_b))   # [G,EMB]

Algebraic restructuring (exact up to fp rounding):
  softmax weights per segment sum to 1, so
      out[g] = (U[g]/D[g]) @ attn_w + attn_b
  with U[g] = sum_{i in g} e_i * values_i, D[g] = sum_{i in g} e_i,
  e_i = exp(values_i . gate_w).  gate_b and the per-segment max shift
  cancel in the U/D ratio (|s| <= ~4 for this data, exp can't overflow).

Device-side structure (v3):
  * gate_w is folded into the values on the host (v_g = v * gate,
    compensated by attn_w' = attn_w / gate[:,None]), so the gate dot
    product becomes a plain row-sum.
  * the row-sum s runs as a pairwise bf16 add-tree on the DVE
    (halving widths 128..1, ~2x faster than tensor_reduce, measured)
    for most blocks, and as ACT activation+accum for the rest -
    split so neither engine is the bottleneck.
  * one-hot P (host-precomputed, bf16, 32 wide) is scaled by e per
    GROUP: E = broadcast-copy of e_g to [128,(16,32)] on ACT, then one
    DVE tensor_tensor multiply -> pe_grp [128, 512].  ~55ns/block vs
    ~700ns for per-block tensor_scalar.
  * windows are SEGW=32 segments; per 128-node block ONE bf16 matmul
    uw[32, 257] += pe.T @ v_ext.  The 257th (ones) column yields D.
    M=32 outputs measurably issue as fast as M=16 (207ns vs 213ns),
    halving PE time per segment.
  * P rides in the same DMA row as v (one 9248B/partition DMA per
    16-block group).

Sharding: indices are sorted, so each of the 8 cores owns G/8 contiguous
segments and their (contiguous) nodes. No collectives.
Window epilogue: u_sb = ACT copy of uw; two PE transposes stage U.T into
[emb, seg] tiles; the D column goes to a [32, W] column stage, then one
PE transpose + DRAM round-trip makes the D row (for the attn_b rank-1
term) and the per-partition D (for 1/D).  Final per 128-segment group:
    Z = U_g @ attn_w' + D * attn_b   (3 f32 matmuls)
    out = Z * (1/max(D,eps))         (ACT per-partition scale)
Empty segments give U=0, D=0 -> out row 0, matching segment_sum.
"""

import numpy as np

P = 128
EMB = 256
EMBX = EMB + 1      # +1 ones column -> D rides the same matmul
HALF = 128
SEGW = 32           # segments per window == one-hot width
NCORES = 8
BLK_PER_DMA = 32    # blocks per DMA group
GRP = 128           # segments per final-matmul group
VCOLS = BLK_PER_DMA * EMBX            # 4112
PCOLS = BLK_PER_DMA * SEGW            # 512
ROWC = VCOLS + PCOLS                  # 4624 cols per group row

# s row-sum engine split per 16-block DMA group:
# blocks [0:NDVE) on the DVE add-tree, [NDVE:16) on ACT accum.
NDVE_H = 13   # tree blocks per 16-block half

_CACHE = {}


# ----------------------------------------------------------------------------
# Host-side preparation: shard + pad nodes into (core, window, block) layout.
# ----------------------------------------------------------------------------
def prepare_host(values, indices, G, gate_w):
    import ml_dtypes

    bf16 = ml_dtypes.bfloat16
    idx = np.ascontiguousarray(np.asarray(indices).astype(np.int64))
    counts = np.bincount(idx, minlength=G)
    seg_start = np.zeros(G + 1, dtype=np.int64)
    np.cumsum(counts, out=seg_start[1:])

    assert G % NCORES == 0
    spc = G // NCORES                      # segments per core
    win_lo = list(range(0, spc, SEGW))     # window seg offsets within a core
    win_w = [min(SEGW, spc - lo) for lo in win_lo]
    W = len(win_lo)

    # blocks per window index = max over cores (SPMD: one program, 8 cores)
    b_w = []
    for w in range(W):
        need = 1
        for c in range(NCORES):
            s0 = c * spc + win_lo[w]
            n = int(seg_start[s0 + win_w[w]] - seg_start[s0])
            need = max(need, (n + P - 1) // P)
        b_w.append(need)
    nblk = sum(b_w)

    gate = np.asarray(gate_w, np.float32).reshape(EMB)

    n_dma = (nblk + BLK_PER_DMA - 1) // BLK_PER_DMA
    nblk_pad = n_dma * BLK_PER_DMA
    vals = np.asarray(values, dtype=np.float32)
    per_core = []
    for c in range(NCORES):
        v_pad = np.zeros((nblk_pad, P, EMBX), dtype=bf16)
        p_pad = np.zeros((nblk_pad, P, SEGW), dtype=bf16)
        gb = 0
        for w in range(W):
            s0 = c * spc + win_lo[w]
            lo = int(seg_start[s0])
            hi = int(seg_start[s0 + win_w[w]])
            r = lo
            for b in range(b_w[w]):
                n = min(P, hi - r)
                if n > 0:
                    v_pad[gb, :n, 0:EMB] = (
                        vals[r : r + n] * gate[None, :]
                    ).astype(bf16)
                    v_pad[gb, :n, EMB] = bf16(1.0)
                    il = (idx[r : r + n] - s0).astype(np.int64)
                    p_pad[gb, np.arange(n), il] = bf16(1.0)
                r += n
                gb += 1
        assert r == hi if W else True
        # regroup: per group row = [16 blocks x 257 v-cols | 16 x 32 P-cols],
        # per-partition contiguous 9248B runs.
        va = np.ascontiguousarray(
            v_pad.reshape(n_dma, BLK_PER_DMA, P, EMBX).transpose(0, 2, 1, 3)
        ).reshape(n_dma, P, VCOLS)
        pa = np.ascontiguousarray(
            p_pad.reshape(n_dma, BLK_PER_DMA, P, SEGW).transpose(0, 2, 1, 3)
        ).reshape(n_dma, P, PCOLS)
        full = np.concatenate([va, pa], axis=2).reshape(n_dma * P, ROWC)
        per_core.append({"v": np.ascontiguousarray(full)})
    meta = {"W": W, "b_w": b_w, "win_lo": win_lo, "win_w": win_w,
            "nblk": nblk, "spc": spc, "n_dma": n_dma}
    return per_core, meta


# ----------------------------------------------------------------------------
# Bass program (identical for all cores; data differs per core).
# ----------------------------------------------------------------------------
def build_bass(meta, reps=1):
    import concourse.bass as bass
    import concourse.bacc as bacc
    import concourse.tile as tile
    from concourse import mybir
    from contextlib import ExitStack

    f32 = mybir.dt.float32
    bf16 = mybir.dt.bfloat16
    Alu = mybir.AluOpType
    Act = mybir.ActivationFunctionType

    W = meta["W"]
    b_w = meta["b_w"]
    win_lo = meta["win_lo"]
    win_w = meta["win_w"]
    nblk = meta["nblk"]
    spc = meta["spc"]
    n_grp = (spc + GRP - 1) // GRP
    assert n_grp * GRP == spc, "final groups assumed exact"

    n_dma = meta["n_dma"]
    nc = bacc.Bacc(
        "TRN2",
        target_bir_lowering=False,
        debug=False,
        enable_asserts=False,
        num_devices=NCORES,
    )

    v_d = nc.dram_tensor("v", [n_dma * P, ROWC], bf16,
                         kind="ExternalInput").ap()
    attn_d = nc.dram_tensor("attn_w", [EMB, EMB], f32, kind="ExternalInput").ap()
    attnb_d = nc.dram_tensor("attn_b", [1, EMB], f32, kind="ExternalInput").ap()
    ident_d = nc.dram_tensor("ident", [P, P], f32, kind="ExternalInput").ap()
    out_d = nc.dram_tensor("out", [spc, EMB], f32, kind="ExternalOutput").ap()

    with ExitStack() as ctx:
        tc = ctx.enter_context(tile.TileContext(nc))
        const = ctx.enter_context(tc.tile_pool(name="const", bufs=1))
        vpool = ctx.enter_context(tc.tile_pool(name="vpool", bufs=5))
        sepool = ctx.enter_context(tc.tile_pool(name="sepool", bufs=4))
        scrpool = ctx.enter_context(tc.tile_pool(name="scrpool", bufs=4))
        trpool = ctx.enter_context(tc.tile_pool(name="trpool", bufs=2))
        pepool = ctx.enter_context(tc.tile_pool(name="pepool", bufs=4))
        opool = ctx.enter_context(tc.tile_pool(name="opool", bufs=2))
        dram = ctx.enter_context(tc.tile_pool(name="dram", bufs=1, space="DRAM"))
        psum2 = ctx.enter_context(tc.tile_pool(name="psum2", bufs=2, space="PSUM"))
        psum3 = ctx.enter_context(tc.tile_pool(name="psum3", bufs=1, space="PSUM"))
        psum1 = ctx.enter_context(tc.tile_pool(name="psum1", bufs=1, space="PSUM"))
        stpool = ctx.enter_context(tc.tile_pool(name="stpool", bufs=2))

        # ---- constants ----
        attn0_sb = const.tile([P, EMB], f32, tag="attn0")
        nc.sync.dma_start(out=attn0_sb, in_=attn_d[0:HALF, :])
        attn1_sb = const.tile([P, EMB], f32, tag="attn1")
        nc.sync.dma_start(out=attn1_sb, in_=attn_d[HALF:EMB, :])
        attnb_sb = const.tile([1, EMB], f32)
        nc.sync.dma_start(out=attnb_sb, in_=attnb_d)
        ident_sb = const.tile([P, P], f32)
        nc.sync.dma_start(out=ident_sb, in_=ident_d)

        u_stage0 = const.tile([P, spc], f32, tag="u_stage0")
        u_stage1 = const.tile([P, spc], f32, tag="u_stage1")
        d_colstage = const.tile([P, W], f32, tag="d_colstage")

        def one_pass():
            vt_tiles = [None] * n_dma
            pe_tiles = [None] * n_dma

            def ensure_group(g):
                if vt_tiles[g] is not None:
                    return
                vt = vpool.tile([P, ROWC], bf16, tag="vt")
                nc.sync.dma_start(out=vt, in_=v_d[g * P : (g + 1) * P, :])
                v3 = vt[:, 0:VCOLS].rearrange("p (n d) -> p n d", d=EMBX)
                s_g = sepool.tile([P, BLK_PER_DMA], f32, tag="s_g")
                e_g = sepool.tile([P, BLK_PER_DMA], f32, tag="e_g")
                E_g = scrpool.tile([P, BLK_PER_DMA, SEGW], bf16, tag="E_g")
                pe_t = pepool.tile([P, PCOLS], bf16, tag="pe_t")
                # two 16-block half-chains to halve DMA->matmul latency
                for h, htag in ((0, "a"), (1, "b")):
                    j0 = h * 16
                    jD = j0 + NDVE_H       # tree blocks [j0:jD)
                    t1 = trpool.tile([P, NDVE_H, 128], bf16, tag="t1" + htag)
                    nc.vector.tensor_tensor(out=t1, in0=v3[:, j0:jD, 0:128],
                                            in1=v3[:, j0:jD, 128:256],
                                            op=Alu.add)
                    t2 = trpool.tile([P, NDVE_H, 64], bf16, tag="t2" + htag)
                    nc.vector.tensor_tensor(out=t2, in0=t1[:, :, 0:64],
                                            in1=t1[:, :, 64:128], op=Alu.add)
                    t3 = trpool.tile([P, NDVE_H, 32], bf16, tag="t3" + htag)
                    nc.vector.tensor_tensor(out=t3, in0=t2[:, :, 0:32],
                                            in1=t2[:, :, 32:64], op=Alu.add)
                    t4 = trpool.tile([P, NDVE_H, 16], bf16, tag="t4" + htag)
                    nc.vector.tensor_tensor(out=t4, in0=t3[:, :, 0:16],
                                            in1=t3[:, :, 16:32], op=Alu.add)
                    t5 = trpool.tile([P, NDVE_H, 8], bf16, tag="t5" + htag)
                    nc.vector.tensor_tensor(out=t5, in0=t4[:, :, 0:8],
                                            in1=t4[:, :, 8:16], op=Alu.add)
                    t6 = trpool.tile([P, NDVE_H, 4], f32, tag="t6" + htag)
                    nc.vector.tensor_tensor(out=t6, in0=t5[:, :, 0:4],
                                            in1=t5[:, :, 4:8], op=Alu.add)
                    t7 = trpool.tile([P, NDVE_H, 2], f32, tag="t7" + htag)
                    nc.vector.tensor_tensor(out=t7, in0=t6[:, :, 0:2],
                                            in1=t6[:, :, 2:4], op=Alu.add)
                    nc.vector.tensor_tensor(
                        out=s_g[:, j0:jD].unsqueeze(2),
                        in0=t7[:, :, 0:1], in1=t7[:, :, 1:2], op=Alu.add)
                    # ACT accum for the rest (exclude the ones column!)
                    for j in range(jD, j0 + 16):
                        scr_a = scrpool.tile([P, EMB], bf16, tag="scr_a")
                        nc.scalar.activation(
                            out=scr_a, in_=vt[:, j * EMBX : j * EMBX + EMB],
                            func=Act.Copy, accum_out=s_g[:, j : j + 1])
                    nc.scalar.activation(e_g[:, j0 : j0 + 16],
                                         s_g[:, j0 : j0 + 16], Act.Exp)
                    nc.scalar.activation(
                        out=E_g[:, j0 : j0 + 16, :],
                        in_=e_g[:, j0 : j0 + 16].unsqueeze(2)
                            .broadcast_to([P, 16, SEGW]),
                        func=Act.Copy)
                    nc.vector.tensor_tensor(
                        out=pe_t[:, j0 * SEGW : (j0 + 16) * SEGW],
                        in0=vt[:, VCOLS + j0 * SEGW : VCOLS + (j0 + 16) * SEGW],
                        in1=E_g[:, j0 : j0 + 16, :].rearrange(
                            "p a b -> p (a b)"),
                        op=Alu.mult)
                vt_tiles[g] = vt
                pe_tiles[g] = pe_t

            gb = 0
            for w in range(W):
                segw = win_w[w]
                uw = psum2.tile([SEGW, EMBX], f32, tag="uw")
                for b in range(b_w[w]):
                    g, j = divmod(gb, BLK_PER_DMA)
                    ensure_group(g)
                    vt = vt_tiles[g]
                    pe_t = pe_tiles[g]
                    nc.tensor.matmul(
                        uw, lhsT=pe_t[:, j * SEGW : (j + 1) * SEGW],
                        rhs=vt[:, j * EMBX : (j + 1) * EMBX],
                        start=(b == 0), stop=(b == b_w[w] - 1))
                    gb += 1
                # ---- window epilogue ----
                off = win_lo[w]
                u_sb = stpool.tile([SEGW, EMBX], f32, tag="u_sb")
                nc.scalar.copy(u_sb, uw)
                t0p = psum3.tile([P, SEGW], f32, tag="t0p")
                nc.tensor.transpose(t0p, u_sb[:, 0:HALF],
                                    ident_sb[0:SEGW, 0:SEGW])
                t1p = psum3.tile([P, SEGW], f32, tag="t1p")
                nc.tensor.transpose(t1p, u_sb[:, HALF:EMB],
                                    ident_sb[0:SEGW, 0:SEGW])
                nc.vector.tensor_copy(u_stage0[:, off : off + segw],
                                      t0p[:, 0:segw])
                nc.vector.tensor_copy(u_stage1[:, off : off + segw],
                                      t1p[:, 0:segw])
                nc.vector.tensor_copy(d_colstage[0:segw, w : w + 1],
                                      u_sb[0:segw, EMB : EMB + 1])

            # ---- D: [SEGW, W] cols -> PE transpose -> DRAM row -> layouts ----
            dwp = psum1.tile([P, SEGW], f32, tag="dwp")
            nc.tensor.transpose(dwp[0:W, 0:SEGW], d_colstage[0:SEGW, 0:W],
                                ident_sb[0:SEGW, 0:SEGW])
            d_wsb = const.tile([P, SEGW], f32, tag="d_wsb")
            nc.scalar.copy(d_wsb[0:W, 0:SEGW], dwp[0:W, 0:SEGW])
            d_dram = dram.tile([1, spc], f32, tag="d_dram")
            nc.sync.dma_start(
                out=d_dram.rearrange("o (a b) -> a (o b)", b=SEGW),
                in_=d_wsb[0:W, 0:SEGW])
            d_row = const.tile([1, spc], f32, tag="d_row")
            nc.sync.dma_start(out=d_row, in_=d_dram)
            d_sq = const.tile([P, GRP], f32, tag="d_sq")
            nc.vector.memset(d_sq, 0.0)
            nc.sync.dma_start(
                out=d_sq[0:n_grp, :],
                in_=d_dram.rearrange("o (g p) -> (o g) p", p=GRP))
            dTp = psum1.tile([P, P], f32, tag="dTp")
            nc.tensor.transpose(dTp, d_sq, ident_sb)
            d_cols = const.tile([P, n_grp], f32, tag="d_cols")
            nc.vector.tensor_copy(d_cols, dTp[:, 0:n_grp])
            d_cl = const.tile([P, n_grp], f32, tag="d_cl")
            nc.vector.tensor_scalar_max(d_cl, d_cols, 1e-30)
            rec = const.tile([P, n_grp], f32, tag="rec")
            nc.vector.reciprocal(rec, d_cl)

            # ---- final: Z = U_g @ attn_w' + D * attn_b, out = Z / D ----
            for g in range(n_grp):
                lo = g * GRP
                z = psum1.tile([GRP, EMB], f32, tag="z")
                nc.tensor.matmul(z, lhsT=u_stage0[:, lo : lo + GRP],
                                 rhs=attn0_sb, start=True, stop=False)
                nc.tensor.matmul(z, lhsT=u_stage1[:, lo : lo + GRP],
                                 rhs=attn1_sb, start=False, stop=False)
                nc.tensor.matmul(z, lhsT=d_row[0:1, lo : lo + GRP],
                                 rhs=attnb_sb, start=False, stop=True)
                o_sb = opool.tile([GRP, EMB], f32, tag="o_sb")
                nc.scalar.activation(o_sb, z, Act.Copy,
                                     scale=rec[:, g : g + 1])
                nc.sync.dma_start(out=out_d[lo : lo + GRP, :], in_=o_sb)

        for _rep in range(reps):
            one_pass()

    nc.compile()
    return nc


def _get_program(meta):
    key = (meta["W"], tuple(meta["b_w"]), tuple(meta["win_lo"]),
           tuple(meta["win_w"]), meta["spc"])
    if key not in _CACHE:
        _CACHE[key] = build_bass(meta)
    return _CACHE[key]


def make_const_inputs(gate_w, attn_w, attn_b):
    gate = np.asarray(gate_w, np.float32).reshape(EMB)
    attn_adj = (np.asarray(attn_w, np.float32)
                / gate[:, None]).astype(np.float32)
    return {
        "attn_w": attn_adj,
        "attn_b": np.asarray(attn_b, np.float32).reshape(1, EMB),
        "ident": np.eye(P, dtype=np.float32),
    }


def build_in_maps(values, indices, num_graphs, gate_w, attn_w, attn_b):
    G = int(num_graphs)
    per_core, meta = prepare_host(np.asarray(values, np.float32), indices, G,
                                  gate_w)
    consts = make_const_inputs(gate_w, attn_w, attn_b)
    in_maps = [{**consts, "v": pc["v"]} for pc in per_core]
    return in_maps, meta


# ----------------------------------------------------------------------------
# Public entry point.
# ----------------------------------------------------------------------------
def kernel(values, indices, num_graphs, gate_w, gate_b, attn_w, attn_b):
    from concourse.bass_utils import run_bass_kernel_spmd

    in_maps, meta = build_in_maps(values, indices, num_graphs,
                                  gate_w, attn_w, attn_b)
    nc = _get_program(meta)
    res = run_bass_kernel_spmd(nc, in_maps, core_ids=list(range(NCORES)))
    out = np.concatenate([res.results[c]["out"] for c in range(NCORES)], axis=0)
    return out[: int(num_graphs)]


# revision 13
# speedup vs baseline: 1.1593x; 1.1593x over previous
"""Trainium2 Bass kernel for AttentionalAggregation (segment softmax-weighted sum).

reference math:
    s = values @ gate_w + gate_b            # [N,1]
    w = segment_softmax(s, indices)         # [N,1]
    out = segment_sum(w * (values @ attn_w + attn_b))   # [G,EMB]

Algebraic restructuring (exact up to fp rounding):
  softmax weights per segment sum to 1, so
      out[g] = (U[g]/D[g]) @ attn_w + attn_b
  with U[g] = sum_{i in g} e_i * values_i, D[g] = sum_{i in g} e_i,
  e_i = exp(values_i . gate_w).  gate_b and the per-segment max shift
  cancel in the U/D ratio (|s| <= ~4 for this data, exp can't overflow).

Device-side structure (v3):
  * gate_w is folded into the values on the host (v_g = v * gate,
    compensated by attn_w' = attn_w / gate[:,None]), so the gate dot
    product becomes a plain row-sum.
  * the row-sum s runs as a pairwise bf16 add-tree on the DVE
    (halving widths 128..1, ~2x faster than tensor_reduce, measured)
    for most blocks, and as ACT activation+accum for the rest -
    split so neither engine is the bottleneck.
  * one-hot P (host-precomputed, bf16, 32 wide) is scaled by e per
    GROUP: E = broadcast-copy of e_g to [128,(16,32)] on ACT, then one
    DVE tensor_tensor multiply -> pe_grp [128, 512].  ~55ns/block vs
    ~700ns for per-block tensor_scalar.
  * windows are SEGW=32 segments; per 128-node block ONE bf16 matmul
    uw[32, 257] += pe.T @ v_ext.  The 257th (ones) column yields D.
    M=32 outputs measurably issue as fast as M=16 (207ns vs 213ns),
    halving PE time per segment.
  * P rides in the same DMA row as v (one 9248B/partition DMA per
    16-block group).

Sharding: indices are sorted, so each of the 8 cores owns G/8 contiguous
segments and their (contiguous) nodes. No collectives.
Window epilogue: u_sb = ACT copy of uw; two PE transposes stage U.T into
[emb, seg] tiles; the D column goes to a [32, W] column stage, then one
PE transpose + DRAM round-trip makes the D row (for the attn_b rank-1
term) and the per-partition D (for 1/D).  Final per 128-segment group:
    Z = U_g @ attn_w' + D * attn_b   (3 f32 matmuls)
    out = Z * (1/max(D,eps))         (ACT per-partition scale)
Empty segments give U=0, D=0 -> out row 0, matching segment_sum.
"""

import numpy as np

P = 128
EMB = 256
EMBX = EMB + 1      # +1 ones column -> D rides the same matmul
HALF = 128
SEGW = 32           # segments per window == one-hot width
NCORES = 8
BLK_PER_DMA = 16    # blocks per DMA group
GRP = 128           # segments per final-matmul group
VCOLS = BLK_PER_DMA * EMBX            # 4112
PCOLS = BLK_PER_DMA * SEGW            # 512
ROWC = VCOLS + PCOLS                  # 4624 cols per group row

# s row-sum engine split per 16-block DMA group:
# blocks [0:NDVE) on the DVE add-tree, [NDVE:16) on ACT accum.
NDVE_H = 13   # tree blocks per 16-block half

_CACHE = {}


# ----------------------------------------------------------------------------
# Host-side preparation: shard + pad nodes into (core, window, block) layout.
# ----------------------------------------------------------------------------
def prepare_host(values, indices, G, gate_w):
    import ml_dtypes

    bf16 = ml_dtypes.bfloat16
    idx = np.ascontiguousarray(np.asarray(indices).astype(np.int64))
    counts = np.bincount(idx, minlength=G)
    seg_start = np.zeros(G + 1, dtype=np.int64)
    np.cumsum(counts, out=seg_start[1:])

    assert G % NCORES == 0
    spc = G // NCORES                      # segments per core
    win_lo = list(range(0, spc, SEGW))     # window seg offsets within a core
    win_w = [min(SEGW, spc - lo) for lo in win_lo]
    W = len(win_lo)

    # blocks per window index = max over cores (SPMD: one program, 8 cores)
    b_w = []
    for w in range(W):
        need = 1
        for c in range(NCORES):
            s0 = c * spc + win_lo[w]
            n = int(seg_start[s0 + win_w[w]] - seg_start[s0])
            need = max(need, (n + P - 1) // P)
        b_w.append(need)
    nblk = sum(b_w)

    gate = np.asarray(gate_w, np.float32).reshape(EMB)

    n_dma = (nblk + BLK_PER_DMA - 1) // BLK_PER_DMA
    nblk_pad = n_dma * BLK_PER_DMA
    vals = np.asarray(values, dtype=np.float32)
    per_core = []
    for c in range(NCORES):
        v_pad = np.zeros((nblk_pad, P, EMBX), dtype=bf16)
        p_pad = np.zeros((nblk_pad, P, SEGW), dtype=bf16)
        gb = 0
        for w in range(W):
            s0 = c * spc + win_lo[w]
            lo = int(seg_start[s0])
            hi = int(seg_start[s0 + win_w[w]])
            r = lo
            for b in range(b_w[w]):
                n = min(P, hi - r)
                if n > 0:
                    v_pad[gb, :n, 0:EMB] = (
                        vals[r : r + n] * gate[None, :]
                    ).astype(bf16)
                    v_pad[gb, :n, EMB] = bf16(1.0)
                    il = (idx[r : r + n] - s0).astype(np.int64)
                    p_pad[gb, np.arange(n), il] = bf16(1.0)
                r += n
                gb += 1
        assert r == hi if W else True
        # regroup: per group row = [16 blocks x 257 v-cols | 16 x 32 P-cols],
        # per-partition contiguous 9248B runs.
        va = np.ascontiguousarray(
            v_pad.reshape(n_dma, BLK_PER_DMA, P, EMBX).transpose(0, 2, 1, 3)
        ).reshape(n_dma, P, VCOLS)
        pa = np.ascontiguousarray(
            p_pad.reshape(n_dma, BLK_PER_DMA, P, SEGW).transpose(0, 2, 1, 3)
        ).reshape(n_dma, P, PCOLS)
        full = np.concatenate([va, pa], axis=2).reshape(n_dma * P, ROWC)
        per_core.append({"v": np.ascontiguousarray(full)})
    meta = {"W": W, "b_w": b_w, "win_lo": win_lo, "win_w": win_w,
            "nblk": nblk, "spc": spc, "n_dma": n_dma}
    return per_core, meta


# ----------------------------------------------------------------------------
# Bass program (identical for all cores; data differs per core).
# ----------------------------------------------------------------------------
def build_bass(meta, reps=1):
    import concourse.bass as bass
    import concourse.bacc as bacc
    import concourse.tile as tile
    from concourse import mybir
    from contextlib import ExitStack

    f32 = mybir.dt.float32
    bf16 = mybir.dt.bfloat16
    Alu = mybir.AluOpType
    Act = mybir.ActivationFunctionType

    W = meta["W"]
    b_w = meta["b_w"]
    win_lo = meta["win_lo"]
    win_w = meta["win_w"]
    nblk = meta["nblk"]
    spc = meta["spc"]
    n_grp = (spc + GRP - 1) // GRP
    assert n_grp * GRP == spc, "final groups assumed exact"

    n_dma = meta["n_dma"]
    nc = bacc.Bacc(
        "TRN2",
        target_bir_lowering=False,
        debug=False,
        enable_asserts=False,
        num_devices=NCORES,
    )

    v_d = nc.dram_tensor("v", [n_dma * P, ROWC], bf16,
                         kind="ExternalInput").ap()
    attn_d = nc.dram_tensor("attn_w", [EMB, EMB], f32, kind="ExternalInput").ap()
    attnb_d = nc.dram_tensor("attn_b", [1, EMB], f32, kind="ExternalInput").ap()
    ident_d = nc.dram_tensor("ident", [P, P], f32, kind="ExternalInput").ap()
    out_d = nc.dram_tensor("out", [spc, EMB], f32, kind="ExternalOutput").ap()

    with ExitStack() as ctx:
        tc = ctx.enter_context(tile.TileContext(nc))
        const = ctx.enter_context(tc.tile_pool(name="const", bufs=1))
        vpool = ctx.enter_context(tc.tile_pool(name="vpool", bufs=8))
        sepool = ctx.enter_context(tc.tile_pool(name="sepool", bufs=4))
        scrpool = ctx.enter_context(tc.tile_pool(name="scrpool", bufs=4))
        trpool = ctx.enter_context(tc.tile_pool(name="trpool", bufs=2))
        pepool = ctx.enter_context(tc.tile_pool(name="pepool", bufs=4))
        opool = ctx.enter_context(tc.tile_pool(name="opool", bufs=2))
        dram = ctx.enter_context(tc.tile_pool(name="dram", bufs=1, space="DRAM"))
        psum2 = ctx.enter_context(tc.tile_pool(name="psum2", bufs=2, space="PSUM"))
        psum3 = ctx.enter_context(tc.tile_pool(name="psum3", bufs=1, space="PSUM"))
        psum1 = ctx.enter_context(tc.tile_pool(name="psum1", bufs=1, space="PSUM"))
        stpool = ctx.enter_context(tc.tile_pool(name="stpool", bufs=2))

        # ---- constants ----
        attn0_sb = const.tile([P, EMB], f32, tag="attn0")
        nc.sync.dma_start(out=attn0_sb, in_=attn_d[0:HALF, :])
        attn1_sb = const.tile([P, EMB], f32, tag="attn1")
        nc.sync.dma_start(out=attn1_sb, in_=attn_d[HALF:EMB, :])
        attnb_sb = const.tile([1, EMB], f32)
        nc.sync.dma_start(out=attnb_sb, in_=attnb_d)
        ident_sb = const.tile([P, P], f32)
        nc.sync.dma_start(out=ident_sb, in_=ident_d)

        u_stage0 = const.tile([P, spc], f32, tag="u_stage0")
        u_stage1 = const.tile([P, spc], f32, tag="u_stage1")
        d_colstage = const.tile([P, W], f32, tag="d_colstage")

        def one_pass():
            vt_tiles = [None] * n_dma
            pe_tiles = [None] * n_dma

            def ensure_group(g):
                if vt_tiles[g] is not None:
                    return
                vt = vpool.tile([P, ROWC], bf16, tag="vt")
                eng = nc.sync if g % 2 == 0 else nc.gpsimd
                eng.dma_start(out=vt, in_=v_d[g * P : (g + 1) * P, :])
                v3 = vt[:, 0:VCOLS].rearrange("p (n d) -> p n d", d=EMBX)
                s_g = sepool.tile([P, BLK_PER_DMA], f32, tag="s_g")
                e_g = sepool.tile([P, BLK_PER_DMA], f32, tag="e_g")
                E_g = scrpool.tile([P, BLK_PER_DMA, SEGW], bf16, tag="E_g")
                pe_t = pepool.tile([P, PCOLS], bf16, tag="pe_t")
                # two 16-block half-chains to halve DMA->matmul latency
                for h, htag in tuple((hh, "ab"[hh]) for hh in range(BLK_PER_DMA // 16)):
                    j0 = h * 16
                    jD = j0 + NDVE_H       # tree blocks [j0:jD)
                    t1 = trpool.tile([P, NDVE_H, 128], bf16, tag="t1" + htag)
                    nc.vector.tensor_tensor(out=t1, in0=v3[:, j0:jD, 0:128],
                                            in1=v3[:, j0:jD, 128:256],
                                            op=Alu.add)
                    t2 = trpool.tile([P, NDVE_H, 64], bf16, tag="t2" + htag)
                    nc.vector.tensor_tensor(out=t2, in0=t1[:, :, 0:64],
                                            in1=t1[:, :, 64:128], op=Alu.add)
                    t3 = trpool.tile([P, NDVE_H, 32], bf16, tag="t3" + htag)
                    nc.vector.tensor_tensor(out=t3, in0=t2[:, :, 0:32],
                                            in1=t2[:, :, 32:64], op=Alu.add)
                    t4 = trpool.tile([P, NDVE_H, 16], bf16, tag="t4" + htag)
                    nc.vector.tensor_tensor(out=t4, in0=t3[:, :, 0:16],
                                            in1=t3[:, :, 16:32], op=Alu.add)
                    t5 = trpool.tile([P, NDVE_H, 8], bf16, tag="t5" + htag)
                    nc.vector.tensor_tensor(out=t5, in0=t4[:, :, 0:8],
                                            in1=t4[:, :, 8:16], op=Alu.add)
                    t6 = trpool.tile([P, NDVE_H, 4], f32, tag="t6" + htag)
                    nc.vector.tensor_tensor(out=t6, in0=t5[:, :, 0:4],
                                            in1=t5[:, :, 4:8], op=Alu.add)
                    t7 = trpool.tile([P, NDVE_H, 2], f32, tag="t7" + htag)
                    nc.vector.tensor_tensor(out=t7, in0=t6[:, :, 0:2],
                                            in1=t6[:, :, 2:4], op=Alu.add)
                    nc.vector.tensor_tensor(
                        out=s_g[:, j0:jD].unsqueeze(2),
                        in0=t7[:, :, 0:1], in1=t7[:, :, 1:2], op=Alu.add)
                    # ACT accum for the rest (exclude the ones column!)
                    for j in range(jD, j0 + 16):
                        scr_a = scrpool.tile([P, EMB], bf16, tag="scr_a")
                        nc.scalar.activation(
                            out=scr_a, in_=vt[:, j * EMBX : j * EMBX + EMB],
                            func=Act.Copy, accum_out=s_g[:, j : j + 1])
                    nc.scalar.activation(e_g[:, j0 : j0 + 16],
                                         s_g[:, j0 : j0 + 16], Act.Exp)
                    nc.scalar.activation(
                        out=E_g[:, j0 : j0 + 16, :],
                        in_=e_g[:, j0 : j0 + 16].unsqueeze(2)
                            .broadcast_to([P, 16, SEGW]),
                        func=Act.Copy)
                    nc.vector.tensor_tensor(
                        out=pe_t[:, j0 * SEGW : (j0 + 16) * SEGW],
                        in0=vt[:, VCOLS + j0 * SEGW : VCOLS + (j0 + 16) * SEGW],
                        in1=E_g[:, j0 : j0 + 16, :].rearrange(
                            "p a b -> p (a b)"),
                        op=Alu.mult)
                vt_tiles[g] = vt
                pe_tiles[g] = pe_t

            gb = 0
            for w in range(W):
                segw = win_w[w]
                uw = psum2.tile([SEGW, EMBX], f32, tag="uw")
                for b in range(b_w[w]):
                    g, j = divmod(gb, BLK_PER_DMA)
                    ensure_group(g)
                    vt = vt_tiles[g]
                    pe_t = pe_tiles[g]
                    nc.tensor.matmul(
                        uw, lhsT=pe_t[:, j * SEGW : (j + 1) * SEGW],
                        rhs=vt[:, j * EMBX : (j + 1) * EMBX],
                        start=(b == 0), stop=(b == b_w[w] - 1))
                    gb += 1
                # ---- window epilogue ----
                off = win_lo[w]
                u_sb = stpool.tile([SEGW, EMBX], f32, tag="u_sb")
                nc.scalar.copy(u_sb, uw)
                t0p = psum3.tile([P, SEGW], f32, tag="t0p")
                nc.tensor.transpose(t0p, u_sb[:, 0:HALF],
                                    ident_sb[0:SEGW, 0:SEGW])
                t1p = psum3.tile([P, SEGW], f32, tag="t1p")
                nc.tensor.transpose(t1p, u_sb[:, HALF:EMB],
                                    ident_sb[0:SEGW, 0:SEGW])
                nc.vector.tensor_copy(u_stage0[:, off : off + segw],
                                      t0p[:, 0:segw])
                nc.vector.tensor_copy(u_stage1[:, off : off + segw],
                                      t1p[:, 0:segw])
                nc.vector.tensor_copy(d_colstage[0:segw, w : w + 1],
                                      u_sb[0:segw, EMB : EMB + 1])

            # ---- D: [SEGW, W] cols -> PE transpose -> DRAM row -> layouts ----
            dwp = psum1.tile([P, SEGW], f32, tag="dwp")
            nc.tensor.transpose(dwp[0:W, 0:SEGW], d_colstage[0:SEGW, 0:W],
                                ident_sb[0:SEGW, 0:SEGW])
            d_wsb = const.tile([P, SEGW], f32, tag="d_wsb")
            nc.scalar.copy(d_wsb[0:W, 0:SEGW], dwp[0:W, 0:SEGW])
            d_dram = dram.tile([1, spc], f32, tag="d_dram")
            nc.sync.dma_start(
                out=d_dram.rearrange("o (a b) -> a (o b)", b=SEGW),
                in_=d_wsb[0:W, 0:SEGW])
            d_row = const.tile([1, spc], f32, tag="d_row")
            nc.sync.dma_start(out=d_row, in_=d_dram)
            d_sq = const.tile([P, GRP], f32, tag="d_sq")
            nc.vector.memset(d_sq, 0.0)
            nc.sync.dma_start(
                out=d_sq[0:n_grp, :],
                in_=d_dram.rearrange("o (g p) -> (o g) p", p=GRP))
            dTp = psum1.tile([P, P], f32, tag="dTp")
            nc.tensor.transpose(dTp, d_sq, ident_sb)
            d_cols = const.tile([P, n_grp], f32, tag="d_cols")
            nc.vector.tensor_copy(d_cols, dTp[:, 0:n_grp])
            d_cl = const.tile([P, n_grp], f32, tag="d_cl")
            nc.vector.tensor_scalar_max(d_cl, d_cols, 1e-30)
            rec = const.tile([P, n_grp], f32, tag="rec")
            nc.vector.reciprocal(rec, d_cl)

            # ---- final: Z = U_g @ attn_w' + D * attn_b, out = Z / D ----
            for g in range(n_grp):
                lo = g * GRP
                z = psum1.tile([GRP, EMB], f32, tag="z")
                nc.tensor.matmul(z, lhsT=u_stage0[:, lo : lo + GRP],
                                 rhs=attn0_sb, start=True, stop=False)
                nc.tensor.matmul(z, lhsT=u_stage1[:, lo : lo + GRP],
                                 rhs=attn1_sb, start=False, stop=False)
                nc.tensor.matmul(z, lhsT=d_row[0:1, lo : lo + GRP],
                                 rhs=attnb_sb, start=False, stop=True)
                o_sb = opool.tile([GRP, EMB], f32, tag="o_sb")
                nc.scalar.activation(o_sb, z, Act.Copy,
                                     scale=rec[:, g : g + 1])
                nc.sync.dma_start(out=out_d[lo : lo + GRP, :], in_=o_sb)

        for _rep in range(reps):
            one_pass()

    nc.compile()
    return nc


def _get_program(meta):
    key = (meta["W"], tuple(meta["b_w"]), tuple(meta["win_lo"]),
           tuple(meta["win_w"]), meta["spc"])
    if key not in _CACHE:
        _CACHE[key] = build_bass(meta)
    return _CACHE[key]


def make_const_inputs(gate_w, attn_w, attn_b):
    gate = np.asarray(gate_w, np.float32).reshape(EMB)
    attn_adj = (np.asarray(attn_w, np.float32)
                / gate[:, None]).astype(np.float32)
    return {
        "attn_w": attn_adj,
        "attn_b": np.asarray(attn_b, np.float32).reshape(1, EMB),
        "ident": np.eye(P, dtype=np.float32),
    }


def build_in_maps(values, indices, num_graphs, gate_w, attn_w, attn_b):
    G = int(num_graphs)
    per_core, meta = prepare_host(np.asarray(values, np.float32), indices, G,
                                  gate_w)
    consts = make_const_inputs(gate_w, attn_w, attn_b)
    in_maps = [{**consts, "v": pc["v"]} for pc in per_core]
    return in_maps, meta


# ----------------------------------------------------------------------------
# Public entry point.
# ----------------------------------------------------------------------------
def kernel(values, indices, num_graphs, gate_w, gate_b, attn_w, attn_b):
    from concourse.bass_utils import run_bass_kernel_spmd

    in_maps, meta = build_in_maps(values, indices, num_graphs,
                                  gate_w, attn_w, attn_b)
    nc = _get_program(meta)
    res = run_bass_kernel_spmd(nc, in_maps, core_ids=list(range(NCORES)))
    out = np.concatenate([res.results[c]["out"] for c in range(NCORES)], axis=0)
    return out[: int(num_graphs)]


# revision 19
# speedup vs baseline: 1.4824x; 1.2787x over previous
"""Trainium2 Bass kernel for AttentionalAggregation (segment softmax-weighted sum).

reference math:
    s = values @ gate_w + gate_b            # [N,1]
    w = segment_softmax(s, indices)         # [N,1]
    out = segment_sum(w * (values @ attn_w + attn_b))   # [G,EMB]

Algebraic restructuring (exact up to fp rounding):
  softmax weights per segment sum to 1, so
      out[g] = (U[g]/D[g]) @ attn_w + attn_b
  with U[g] = sum_{i in g} e_i * values_i, D[g] = sum_{i in g} e_i,
  e_i = exp(values_i . gate_w).  gate_b and the per-segment max shift
  cancel in the U/D ratio (|s| <= ~4 for this data, exp can't overflow).

Device-side structure:
  * gate_w is folded into the values on the host (v_g = v * gate,
    compensated by attn_w' = attn_w / gate[:,None]), so the gate dot
    product becomes a plain row-sum.
  * the row-sum s runs as a pairwise bf16 add-tree on the DVE (widths
    128->64->32, then one tensor_reduce; ~40% faster than a flat
    tensor_reduce, measured) for 13 blocks per 16-block group, and as
    ACT activation+accum for the other 3 - split so neither engine is
    the bottleneck.
  * one-hot P (host-precomputed, bf16, 32 wide) is scaled by e per
    group: E = broadcast-copy of e_g to [128,(16,32)] on ACT, then one
    DVE tensor_tensor multiply -> pe_t [128, 512].
  * windows are SEGW=32 segments; per 128-node block ONE bf16 matmul
    uw[32, 257] += pe.T @ v_ext.  The 257th (ones) column yields D.
  * P rides in the same DMA row as v; group sizes are [4, 12, 16...]
    so the first chain starts after a ~0.3MB transfer, all on the one
    SP HWDGE ring (a single ring measures faster than 2-3 rings, and
    FIFO order means group 0 lands first).
  * group setup is split into start (DMA + tree + ACT-accum) and
    finish (exp + E + pe multiply); the NEXT group's start is emitted
    before the current finish so neither the DVE nor ACT queue is
    head-of-line blocked on the cross-engine e-chain.
  * window epilogues are deferred LAG=2 windows so their PE transposes
    do not stall the streaming matmuls.

Sharding: indices are sorted, so each of the 8 cores owns G/8 contiguous
segments and their (contiguous) nodes. No collectives.
Every 4 windows form a final group staged as u_big [128 segs, 257]
(window w at partitions (w%4)*32, legal 0/32/64/96 bases): column 256
is then the per-partition D (reciprocal -> 1/D scale, PE transpose ->
D row), halves transpose to U.T, and
    Z = U_g @ attn_w' + D * attn_b   (bf16 + f32 rank-1 matmuls)
    out = Z * (1/max(D,eps))         (ACT per-partition scale)
runs interleaved into the stream right after the 4th window.
Empty segments give U=0, D=0 -> out row 0, matching segment_sum.
"""

import numpy as np

P = 128
EMB = 256
EMBX = EMB + 1      # +1 ones column -> D rides the same matmul
HALF = 128
SEGW = 32           # segments per window == one-hot width
NCORES = 8
GRP = 128           # segments per final-matmul group

_CACHE = {}


# ----------------------------------------------------------------------------
# Host-side preparation: shard + pad nodes into (core, window, block) layout.
# ----------------------------------------------------------------------------
def prepare_host(values, indices, G, gate_w):
    import ml_dtypes

    bf16 = ml_dtypes.bfloat16
    idx = np.ascontiguousarray(np.asarray(indices).astype(np.int64))
    counts = np.bincount(idx, minlength=G)
    seg_start = np.zeros(G + 1, dtype=np.int64)
    np.cumsum(counts, out=seg_start[1:])

    assert G % NCORES == 0
    spc = G // NCORES                      # segments per core
    win_lo = list(range(0, spc, SEGW))     # window seg offsets within a core
    win_w = [min(SEGW, spc - lo) for lo in win_lo]
    W = len(win_lo)

    # blocks per window index = max over cores (SPMD: one program, 8 cores)
    b_w = []
    for w in range(W):
        need = 1
        for c in range(NCORES):
            s0 = c * spc + win_lo[w]
            n = int(seg_start[s0 + win_w[w]] - seg_start[s0])
            need = max(need, (n + P - 1) // P)
        b_w.append(need)
    nblk = sum(b_w)

    gate = np.asarray(gate_w, np.float32).reshape(EMB)

    # variable group sizes: tiny first groups -> pipeline starts early,
    # then big 32-block DMAs to amortize per-transfer overhead
    rest = nblk - 16
    n16 = max(0, -(-rest // 16))
    sizes = [4, 12] + [16] * n16
    nblk_pad = sum(sizes)
    n_dma = len(sizes)
    assert nblk_pad >= nblk
    vals = np.asarray(values, dtype=np.float32)
    per_core = []
    for c in range(NCORES):
        v_pad = np.zeros((nblk_pad, P, EMBX), dtype=bf16)
        p_pad = np.zeros((nblk_pad, P, SEGW), dtype=bf16)
        gb = 0
        for w in range(W):
            s0 = c * spc + win_lo[w]
            lo = int(seg_start[s0])
            hi = int(seg_start[s0 + win_w[w]])
            r = lo
            for b in range(b_w[w]):
                n = min(P, hi - r)
                if n > 0:
                    v_pad[gb, :n, 0:EMB] = (
                        vals[r : r + n] * gate[None, :]
                    ).astype(bf16)
                    v_pad[gb, :n, EMB] = bf16(1.0)
                    il = (idx[r : r + n] - s0).astype(np.int64)
                    p_pad[gb, np.arange(n), il] = bf16(1.0)
                r += n
                gb += 1
        assert r == hi if W else True
        # flat layout [P, total]: per group [sz x 257 v-cols | sz x 32 P-cols],
        # per-partition contiguous runs.
        parts = []
        o = 0
        for sz in sizes:
            va = v_pad[o : o + sz].transpose(1, 0, 2).reshape(P, sz * EMBX)
            pa = p_pad[o : o + sz].transpose(1, 0, 2).reshape(P, sz * SEGW)
            parts += [va, pa]
            o += sz
        full = np.ascontiguousarray(np.concatenate(parts, axis=1))
        per_core.append({"v": full})
    meta = {"W": W, "b_w": b_w, "win_lo": win_lo, "win_w": win_w,
            "nblk": nblk, "spc": spc, "n_dma": n_dma,
            "sizes": tuple(sizes)}
    return per_core, meta


# ----------------------------------------------------------------------------
# Bass program (identical for all cores; data differs per core).
# ----------------------------------------------------------------------------
def build_bass(meta, reps=1):
    import concourse.bacc as bacc
    import concourse.tile as tile
    from concourse import mybir
    from contextlib import ExitStack

    f32 = mybir.dt.float32
    bf16 = mybir.dt.bfloat16
    Alu = mybir.AluOpType
    Act = mybir.ActivationFunctionType
    AxX = mybir.AxisListType.X

    W = meta["W"]
    b_w = meta["b_w"]
    win_lo = meta["win_lo"]
    win_w = meta["win_w"]
    nblk = meta["nblk"]
    spc = meta["spc"]
    n_grp = (spc + GRP - 1) // GRP
    assert n_grp * GRP == spc, "final groups assumed exact"

    n_dma = meta["n_dma"]
    sizes = list(meta["sizes"])
    coffs, boffs = [], []
    co = bo = 0
    for sz in sizes:
        coffs.append(co)
        boffs.append(bo)
        co += sz * (EMBX + SEGW)
        bo += sz
    TOT = co
    nc = bacc.Bacc(
        "TRN2",
        target_bir_lowering=False,
        debug=False,
        enable_asserts=False,
        num_devices=NCORES,
    )

    v_d = nc.dram_tensor("v", [P, TOT], bf16,
                         kind="ExternalInput").ap()
    attn_d = nc.dram_tensor("attn_w", [EMB, EMB], bf16, kind="ExternalInput").ap()
    attnb_d = nc.dram_tensor("attn_b", [1, EMB], f32, kind="ExternalInput").ap()
    ident_d = nc.dram_tensor("ident", [P, P], f32, kind="ExternalInput").ap()
    out_d = nc.dram_tensor("out", [spc, EMB], f32, kind="ExternalOutput").ap()

    with ExitStack() as ctx:
        tc = ctx.enter_context(tile.TileContext(nc))
        const = ctx.enter_context(tc.tile_pool(name="const", bufs=1))
        vpool = ctx.enter_context(tc.tile_pool(name="vpool", bufs=8))
        sepool = ctx.enter_context(tc.tile_pool(name="sepool", bufs=4))
        scrpool = ctx.enter_context(tc.tile_pool(name="scrpool", bufs=4))
        trpool = ctx.enter_context(tc.tile_pool(name="trpool", bufs=2))
        pepool = ctx.enter_context(tc.tile_pool(name="pepool", bufs=4))
        opool = ctx.enter_context(tc.tile_pool(name="opool", bufs=2))
        psum2 = ctx.enter_context(tc.tile_pool(name="psum2", bufs=4, space="PSUM"))
        psum3 = ctx.enter_context(tc.tile_pool(name="psum3", bufs=2, space="PSUM"))
        psum1 = ctx.enter_context(tc.tile_pool(name="psum1", bufs=1, space="PSUM"))
        stpool = ctx.enter_context(tc.tile_pool(name="stpool", bufs=2))

        # ---- constants ----
        attn0_sb = const.tile([P, EMB], bf16, tag="attn0")
        nc.scalar.dma_start(out=attn0_sb, in_=attn_d[0:HALF, :])
        attn1_sb = const.tile([P, EMB], bf16, tag="attn1")
        nc.scalar.dma_start(out=attn1_sb, in_=attn_d[HALF:EMB, :])
        attnb_sb = const.tile([1, EMB], f32)
        nc.scalar.dma_start(out=attnb_sb, in_=attnb_d)
        ident_sb = const.tile([P, P], f32)
        nc.scalar.dma_start(out=ident_sb, in_=ident_d)

        assert all(sw == SEGW for sw in win_w) and W % (GRP // SEGW) == 0
        WPG = GRP // SEGW      # windows per final group

        def one_pass():
            vt_tiles = [None] * len(sizes)
            pe_tiles = [None] * len(sizes)

            started = {}

            def start_group(g):
                if g in started:
                    return
                sz = sizes[g]
                vcols = sz * EMBX
                rowc = sz * (EMBX + SEGW)
                nbuf = 8 if sz == 16 else 1
                vt = vpool.tile([P, rowc], bf16, tag=f"vt{g if sz != 16 else ''}",
                                bufs=nbuf)
                nc.sync.dma_start(out=vt, in_=v_d[:, coffs[g] : coffs[g] + rowc])
                v3 = vt[:, 0:vcols].rearrange("p (n d) -> p n d", d=EMBX)
                s_g = sepool.tile([P, sz], f32, tag=f"s_g{sz}")
                e_g = sepool.tile([P, sz], f32, tag=f"e_g{sz}")
                E_g = scrpool.tile([P, sz, SEGW], bf16, tag=f"E_g{sz}")
                pe_t = pepool.tile([P, sz * SEGW], bf16, tag=f"pe_t{sz}",
                                   bufs=nbuf // 2 if nbuf > 1 else 1)
                # 16-block sub-chains bound the DMA->matmul latency
                for j0 in range(0, sz, 16):
                    sub = min(16, sz - j0)
                    nact = (3 * sub) // 16
                    nd = sub - nact
                    jD = j0 + nd
                    t1 = trpool.tile([P, nd, 128], bf16, tag=f"t1_{nd}")
                    nc.vector.tensor_tensor(out=t1, in0=v3[:, j0:jD, 0:128],
                                            in1=v3[:, j0:jD, 128:256],
                                            op=Alu.add)
                    t2 = trpool.tile([P, nd, 64], bf16, tag=f"t2_{nd}")
                    nc.vector.tensor_tensor(out=t2, in0=t1[:, :, 0:64],
                                            in1=t1[:, :, 64:128], op=Alu.add)
                    t3 = trpool.tile([P, nd, 32], bf16, tag=f"t3_{nd}")
                    nc.vector.tensor_tensor(out=t3, in0=t2[:, :, 0:32],
                                            in1=t2[:, :, 32:64], op=Alu.add)
                    nc.vector.tensor_reduce(out=s_g[:, j0:jD], in_=t3,
                                            axis=AxX, op=Alu.add)
                    # ACT accum for the rest (exclude the ones column!)
                    for j in range(jD, j0 + sub):
                        scr_a = scrpool.tile([P, EMB], bf16, tag="scr_a")
                        nc.scalar.activation(
                            out=scr_a, in_=vt[:, j * EMBX : j * EMBX + EMB],
                            func=Act.Copy, accum_out=s_g[:, j : j + 1])
                started[g] = (vt, s_g, e_g, E_g, pe_t)

            def finish_group(g):
                if vt_tiles[g] is not None:
                    return
                vt, s_g, e_g, E_g, pe_t = started[g]
                sz = sizes[g]
                vcols = sz * EMBX
                rowc = sz * (EMBX + SEGW)
                for j0 in range(0, sz, 16):
                    sub = min(16, sz - j0)
                    nc.scalar.activation(e_g[:, j0 : j0 + sub],
                                         s_g[:, j0 : j0 + sub], Act.Exp)
                    nc.scalar.activation(
                        out=E_g[:, j0 : j0 + sub, :],
                        in_=e_g[:, j0 : j0 + sub].unsqueeze(2)
                            .broadcast_to([P, sub, SEGW]),
                        func=Act.Copy)
                    nc.vector.tensor_tensor(
                        out=pe_t[:, j0 * SEGW : (j0 + sub) * SEGW],
                        in0=vt[:, vcols + j0 * SEGW : vcols + (j0 + sub) * SEGW],
                        in1=E_g[:, j0 : j0 + sub, :].rearrange(
                            "p a b -> p (a b)"),
                        op=Alu.mult)
                vt_tiles[g] = vt
                pe_tiles[g] = pe_t

            def ensure_group(g):
                if vt_tiles[g] is not None:
                    return
                start_group(g)
                if g + 1 < len(sizes):
                    start_group(g + 1)
                finish_group(g)

            for g0 in range(5):
                start_group(g0)
            finish_group(0)

            ubig_box = [None]

            def final_group(g, u_big):
                # D is the per-partition column 256 of u_big.
                d_cl = sepool.tile([P, 1], f32, tag="d_cl")
                nc.vector.tensor_scalar_max(d_cl, u_big[:, EMB : EMB + 1],
                                            1e-30)
                rec_g = sepool.tile([P, 1], f32, tag="rec_g")
                nc.vector.reciprocal(rec_g, d_cl)
                drp = psum3.tile([P, P], f32, tag="trp")
                nc.tensor.transpose(drp[0:1, 0:P], u_big[:, EMB : EMB + 1],
                                    ident_sb)
                d_row = sepool.tile([1, P], f32, tag="d_row")
                nc.scalar.copy(d_row, drp[0:1, 0:P])
                t0p = psum3.tile([P, P], f32, tag="trp")
                nc.tensor.transpose(t0p, u_big[:, 0:HALF], ident_sb)
                u0_sb = stpool.tile([P, P], bf16, tag="u0_sb")
                nc.vector.tensor_copy(u0_sb, t0p)
                t1p = psum3.tile([P, P], f32, tag="trp")
                nc.tensor.transpose(t1p, u_big[:, HALF:EMB], ident_sb)
                u1_sb = stpool.tile([P, P], bf16, tag="u1_sb")
                nc.vector.tensor_copy(u1_sb, t1p)
                z = psum1.tile([GRP, EMB], f32, tag="z")
                nc.tensor.matmul(z, lhsT=u0_sb, rhs=attn0_sb,
                                 start=True, stop=False)
                nc.tensor.matmul(z, lhsT=u1_sb, rhs=attn1_sb,
                                 start=False, stop=False)
                nc.tensor.matmul(z, lhsT=d_row, rhs=attnb_sb,
                                 start=False, stop=True)
                o_sb = opool.tile([GRP, EMB], f32, tag="o_sb")
                nc.scalar.activation(o_sb, z, Act.Copy, scale=rec_g)
                lo = g * GRP
                nc.sync.dma_start(out=out_d[lo : lo + GRP, :], in_=o_sb)

            def epilogue(w, uw):
                if w % WPG == 0:
                    u_big_t = stpool.tile([P, EMBX], f32, tag="u_big")
                    ubig_box[0] = u_big_t
                u_big = ubig_box[0]
                po = (w % WPG) * SEGW
                nc.scalar.copy(u_big[po : po + SEGW, :], uw)
                if w % WPG == WPG - 1:
                    final_group(w // WPG, u_big)

            LAG = 2
            pending = []
            gb = 0
            for w in range(W):
                uw = psum2.tile([SEGW, EMBX], f32, tag="uw")
                for b in range(b_w[w]):
                    g = 0
                    while g + 1 < len(sizes) and boffs[g + 1] <= gb:
                        g += 1
                    j = gb - boffs[g]
                    ensure_group(g)
                    vt = vt_tiles[g]
                    pe_t = pe_tiles[g]
                    nc.tensor.matmul(
                        uw, lhsT=pe_t[:, j * SEGW : (j + 1) * SEGW],
                        rhs=vt[:, j * EMBX : (j + 1) * EMBX],
                        start=(b == 0), stop=(b == b_w[w] - 1))
                    gb += 1
                pending.append((w, uw))
                if len(pending) > LAG:
                    epilogue(*pending.pop(0))
            for pw in pending:
                epilogue(*pw)


        for _rep in range(reps):
            one_pass()

    nc.compile()
    return nc


def _get_program(meta):
    key = (meta["W"], tuple(meta["b_w"]), tuple(meta["win_lo"]),
           tuple(meta["win_w"]), meta["spc"])
    if key not in _CACHE:
        _CACHE[key] = build_bass(meta)
    return _CACHE[key]


def make_const_inputs(gate_w, attn_w, attn_b):
    gate = np.asarray(gate_w, np.float32).reshape(EMB)
    import ml_dtypes
    attn_adj = (np.asarray(attn_w, np.float32)
                / gate[:, None]).astype(ml_dtypes.bfloat16)
    return {
        "attn_w": attn_adj,
        "attn_b": np.asarray(attn_b, np.float32).reshape(1, EMB),
        "ident": np.eye(P, dtype=np.float32),
    }


def build_in_maps(values, indices, num_graphs, gate_w, attn_w, attn_b):
    G = int(num_graphs)
    per_core, meta = prepare_host(np.asarray(values, np.float32), indices, G,
                                  gate_w)
    consts = make_const_inputs(gate_w, attn_w, attn_b)
    in_maps = [{**consts, "v": pc["v"]} for pc in per_core]
    return in_maps, meta


# ----------------------------------------------------------------------------
# Public entry point.
# ----------------------------------------------------------------------------
def kernel(values, indices, num_graphs, gate_w, gate_b, attn_w, attn_b):
    from concourse.bass_utils import run_bass_kernel_spmd

    in_maps, meta = build_in_maps(values, indices, num_graphs,
                                  gate_w, attn_w, attn_b)
    nc = _get_program(meta)
    res = run_bass_kernel_spmd(nc, in_maps, core_ids=list(range(NCORES)))
    out = np.concatenate([res.results[c]["out"] for c in range(NCORES)], axis=0)
    return out[: int(num_graphs)]

